# revision 20
# baseline (speedup 1.0000x reference)
"""SPINE model Trainium2 kernel — 8-core SPMD, batch-sharded.

Per core c (rows 1024c..1024c+1023):
  h.T = clamp(W1 @ X.T + b1, 0, 1)   via f32r matmuls, X.T extended with x0
  out = h @ W2.T + b2                natural layout
  row stats for distances-from-row-0 via ones/h0 stationary matmuls (h side)
  and GpSimd/ACT/DVE row reductions (y side); loss partials reduced on device.
Host: concat/transpose outputs, merge per-core top-10 candidates, combine
scalar partials.

The eps cross-terms of torch PairwiseDistance (2*eps*(rs_i - rs_j)) are
dropped: |2*eps*Δrs| ≲ 6e-5 absolute on squared distances of O(100), i.e.
~3e-7 relative on the distances — far below the f32r rounding noise.
The D*eps^2 diagonal term is kept (free).
"""
import numpy as np

B = 8192
NCORES = 8
BL = B // NCORES          # 1024 rows per core
D = 300
H = 1000
OUT = 300
K = 10
EPS = 1e-6
RHO_STAR = 0.15

U_CH = 8                  # hidden chunks
U_SZ = H // U_CH          # 125
D_CHUNKS = [(0, 128), (128, 128), (256, 44)]
EXT = 2                   # x0 column, duplicated for even-N f32r rule
MV_BLOCKS = [(0, 512), (512, 512), (1024, 2)]
I_BLOCKS = 8              # out row blocks of 128

_CACHE = {}


def _round_f32r(x: np.ndarray) -> np.ndarray:
    u = np.ascontiguousarray(x, dtype=np.float32).view(np.uint32)
    r = (u + np.uint32(0x7FF) + ((u >> np.uint32(12)) & np.uint32(1))) & np.uint32(
        0xFFFFF000
    )
    return r.view(np.float32)


def _build(use_f32r=True):
    import concourse.mybir as mybir
    import concourse.tile as tile
    from concourse import bacc

    F32 = mybir.dt.float32
    MMT = mybir.dt.float32r if use_f32r else F32
    AX = mybir.AxisListType
    OP = mybir.AluOpType
    AF = mybir.ActivationFunctionType

    nc = bacc.Bacc("TRN2", target_bir_lowering=False, debug=False,
                   num_devices=NCORES)

    xt = nc.dram_tensor("xt", [D, BL + EXT], MMT, kind="ExternalInput")
    y_in = nc.dram_tensor("y_in", [BL, D], F32, kind="ExternalInput")
    w1t = nc.dram_tensor("w1t", [D, H], MMT, kind="ExternalInput")
    w2t = nc.dram_tensor("w2t", [H, OUT], MMT, kind="ExternalInput")
    b1s = nc.dram_tensor("b1s", [U_SZ, U_CH], F32, kind="ExternalInput")
    b1r = nc.dram_tensor("b1r", [U_SZ, 2 * U_CH], F32, kind="ExternalInput")
    b2 = nc.dram_tensor("b2", [OUT], F32, kind="ExternalInput")
    y0 = nc.dram_tensor("y0", [D], F32, kind="ExternalInput")
    onesd = nc.dram_tensor("onesd", [128, 2], MMT, kind="ExternalInput")

    out_o = nc.dram_tensor("out_o", [BL, OUT], F32, kind="ExternalOutput")
    ht_o = nc.dram_tensor("ht_o", [H, BL], F32, kind="ExternalOutput")
    dy_o = nc.dram_tensor("dy_o", [128, 8], F32, kind="ExternalOutput")
    dh_o = nc.dram_tensor("dh_o", [1, BL + 1], F32, kind="ExternalOutput")
    cs_o = nc.dram_tensor("cs_o", [U_SZ, U_CH], F32, kind="ExternalOutput")
    rec_o = nc.dram_tensor("rec_o", [128, I_BLOCKS], F32, kind="ExternalOutput")
    sc_o = nc.dram_tensor("sc_o", [1, 4], F32, kind="ExternalOutput")

    with tile.TileContext(nc) as tc:
        with (
            tc.tile_pool(name="const", bufs=1) as cp,
            tc.tile_pool(name="ht", bufs=1) as hp,
            tc.tile_pool(name="work", bufs=3) as wp,
            tc.tile_pool(name="small", bufs=1) as sp,
            tc.tile_pool(name="ps_mm1", bufs=2, space="PSUM") as ps1,
            tc.tile_pool(name="ps_out", bufs=2, space="PSUM") as ps2,
            tc.tile_pool(name="ps_ab", bufs=2, space="PSUM") as ps3,
        ):
            # ---- input loads, priority order ----
            xt_sb = cp.tile([128, 3, BL + EXT], MMT)
            w1t_sb = cp.tile([128, 3, H], MMT)
            nc.sync.dma_start(xt_sb[:128, 0, :512], xt[0:128, :512])
            nc.sync.dma_start(w1t_sb[:128, 0, :], w1t[0:128, :])
            nc.sync.dma_start(xt_sb[:128, 0, 512:], xt[0:128, 512:])
            for c, (lo, sz) in list(enumerate(D_CHUNKS))[1:]:
                nc.sync.dma_start(w1t_sb[:sz, c, :], w1t[lo:lo + sz, :])
                nc.sync.dma_start(xt_sb[:sz, c, :], xt[lo:lo + sz, :])
            b1_sb = cp.tile([U_SZ, U_CH], F32)
            nc.sync.dma_start(b1_sb[:], b1s[:])
            b1r_sb = cp.tile([U_SZ, 2 * U_CH], F32)
            nc.sync.dma_start(b1r_sb[:], b1r[:])
            w2t_sb = cp.tile([U_SZ, U_CH, OUT], MMT)
            y_sb = cp.tile([128, I_BLOCKS, D], F32)
            for uc in range(U_CH):
                nc.sync.dma_start(w2t_sb[:, uc, :],
                                  w2t[uc * U_SZ:(uc + 1) * U_SZ, :])
            for b in range(I_BLOCKS):
                nc.sync.dma_start(y_sb[:, b, :], y_in[b * 128:(b + 1) * 128, :])
            b2b_sb = cp.tile([128, OUT], F32)
            nc.sync.dma_start(b2b_sb[:], b2[:].partition_broadcast(128))
            y0b_sb = cp.tile([128, D], F32)
            nc.sync.dma_start(y0b_sb[:], y0[:].partition_broadcast(128))
            ones_sb = sp.tile([128, 2], MMT)
            nc.sync.dma_start(ones_sb[:], onesd[:])

            ht_sb = hp.tile([U_SZ, U_CH, BL + EXT], MMT)
            h2_sb = hp.tile([U_SZ, U_CH, BL + EXT], MMT)

            # preload ACT Square table during input-DMA window
            dum = sp.tile([U_SZ, 2], F32)
            nc.scalar.activation(dum[:], b1_sb[:, 0:2], AF.Square)

            # ---- MM1 main: cols 0..1023 ----
            for ib in range(2):
                lo = ib * 512
                for u in range(U_CH):
                    ps = ps1.tile([U_SZ, 512], F32, tag="mm1")
                    for c, (dlo, dsz) in enumerate(D_CHUNKS):
                        nc.tensor.matmul(
                            ps[:],
                            w1t_sb[:dsz, c, u * U_SZ:(u + 1) * U_SZ],
                            xt_sb[:dsz, c, lo:lo + 512],
                            start=(c == 0),
                            stop=(c == 2),
                        )
                    tmp = wp.tile([U_SZ, 512], F32, tag="mm1tmp")
                    nc.scalar.activation(
                        tmp[:], ps[:], AF.Relu, bias=b1_sb[:, u:u + 1]
                    )
                    nc.vector.tensor_scalar_min(
                        ht_sb[:, u, lo:lo + 512], tmp[:], 1.0
                    )

            # ---- MM1 x0 columns: all u in one [125,16] psum ----
            ps_x0 = ps1.tile([U_SZ, 16], F32, tag="mm1")
            for u in range(U_CH):
                for c, (dlo, dsz) in enumerate(D_CHUNKS):
                    nc.tensor.matmul(
                        ps_x0[:, 2 * u:2 * u + 2],
                        w1t_sb[:dsz, c, u * U_SZ:(u + 1) * U_SZ],
                        xt_sb[:dsz, c, BL:BL + EXT],
                        start=(c == 0),
                        stop=(c == 2),
                    )
            t16 = sp.tile([U_SZ, 16], F32)
            nc.vector.tensor_tensor(t16[:], ps_x0[:], b1r_sb[:], op=OP.add)
            nc.vector.tensor_scalar(
                ht_sb[:, :, BL:BL + EXT], t16[:], 1.0, 0.0,
                op0=OP.min, op1=OP.max,
            )

            # ---- h^2 (ACT / gpsimd split) + x0 cols ----
            for u in range(U_CH):
                if u < 6:
                    nc.scalar.activation(
                        h2_sb[:, u, :BL], ht_sb[:, u, :BL].bitcast(F32),
                        AF.Square,
                    )
                else:
                    nc.gpsimd.tensor_tensor(
                        h2_sb[:, u, :BL], ht_sb[:, u, :BL], ht_sb[:, u, :BL],
                        op=OP.mult,
                    )
            nc.scalar.activation(
                h2_sb[:, :, BL:BL + EXT],
                ht_sb[:, :, BL:BL + EXT].bitcast(F32), AF.Square,
            )

            # ---- h.T out + asl colsums (early) ----
            for u in range(U_CH):
                nc.sync.dma_start(
                    ht_o[u * U_SZ:(u + 1) * U_SZ, :],
                    ht_sb[:, u, :BL].bitcast(F32),
                )
            cs_sb = sp.tile([U_SZ, U_CH], F32)
            nc.vector.tensor_reduce(
                cs_sb[:], ht_sb[:, :, :BL].bitcast(F32), axis=AX.X, op=OP.add
            )
            nc.sync.dma_start(cs_o[:], cs_sb[:])

            # ---- MM2: out = h @ W2.T + b2; rec partials ----
            rec_sb = sp.tile([128, I_BLOCKS], F32)
            for i in range(I_BLOCKS):
                ps = ps2.tile([128, OUT], F32, tag="mm2")
                for u in range(U_CH):
                    nc.tensor.matmul(
                        ps[:],
                        ht_sb[:, u, i * 128:(i + 1) * 128],
                        w2t_sb[:, u, :],
                        start=(u == 0),
                        stop=(u == U_CH - 1),
                    )
                osb = wp.tile([128, OUT], F32, tag="osb")
                nc.vector.tensor_tensor(osb[:], ps[:], b2b_sb[:], op=OP.add)
                nc.sync.dma_start(out_o[i * 128:(i + 1) * 128, :], osb[:])
                e = wp.tile([128, OUT], F32, tag="e")
                nc.gpsimd.tensor_tensor(e[:], osb[:], y_sb[:, i, :],
                                        op=OP.subtract)
                scr = wp.tile([128, OUT], F32, tag="scr")
                nc.scalar.activation(
                    scr[:], e[:], AF.Square, accum_out=rec_sb[:, i:i + 1]
                )
            nc.sync.dma_start(rec_o[:], rec_sb[:])

            # ---- distance stat matmuls: dot_j, sq_j (partition 0) ----
            # dh_o returns d2 partial = sq_j - 2*dot_j (col BL = -sq0);
            # host adds C and takes sqrt.
            dh_sb = sp.tile([1, BL + EXT], F32)
            sc_sb = sp.tile([1, 4], F32)
            nc.vector.memset(sc_sb[:, 0:1], 0.0)
            nc.vector.memset(sc_sb[:, 3:4], 0.0)
            for ib, (lo, n) in enumerate(MV_BLOCKS):
                pdot = ps3.tile([1, n], F32, tag="dot")
                psq = ps3.tile([1, n], F32, tag="sq")
                for u in range(U_CH):
                    st, sp_ = (u == 0), (u == U_CH - 1)
                    nc.tensor.matmul(pdot[:], ht_sb[:, u, BL:BL + 1],
                                     ht_sb[:, u, lo:lo + n], start=st, stop=sp_)
                    nc.tensor.matmul(psq[:], ones_sb[:U_SZ, 0:1],
                                     h2_sb[:, u, lo:lo + n], start=st, stop=sp_)
                if ib < 2:
                    nc.vector.tensor_reduce(sc_sb[:, ib + 1:ib + 2], psq[:],
                                            axis=AX.X, op=OP.add)
                t1 = wp.tile([1, n], F32, tag="d2t1")
                nc.vector.tensor_scalar_mul(t1[:], pdot[:], -2.0)
                nc.vector.tensor_tensor(dh_sb[:, lo:lo + n], t1[:], psq[:],
                                        op=OP.add)
            nc.sync.dma_start(dh_o[:], dh_sb[:, :BL + 1])
            nc.sync.dma_start(sc_o[:], sc_sb[:])

            # ---- y distances (tail-fill; host adds eps^2 + sqrt) ----
            zt = cp.tile([128, I_BLOCKS, D], F32)
            nc.gpsimd.tensor_tensor(
                zt[:], y_sb[:],
                y0b_sb[:, None, :].broadcast_to([128, I_BLOCKS, D]),
                op=OP.subtract,
            )
            sqz = sp.tile([128, 8], F32)
            for b in range(I_BLOCKS):
                zscr = wp.tile([128, D], F32, tag="zscr")
                nc.scalar.activation(zscr[:], zt[:, b, :], AF.Square,
                                     accum_out=sqz[:, b:b + 1])
            nc.sync.dma_start(dy_o[:], sqz[:])


    nc.compile()
    return nc


def _get_nc(use_f32r=True):
    key = ("nc", use_f32r)
    if key not in _CACHE:
        _CACHE[key] = _build(use_f32r)
    return _CACHE[key]


def _make_in_maps(batch_x, batch_y, W1, b1, W2, b2, use_f32r=True):
    rnd = _round_f32r if use_f32r else (
        lambda a: np.ascontiguousarray(a, dtype=np.float32))
    w1t_np = rnd(W1.T)
    w2t_np = rnd(W2.T)
    b1s_np = np.ascontiguousarray(b1.reshape(U_CH, U_SZ).T)
    b1r_np = np.ascontiguousarray(np.repeat(b1s_np, 2, axis=1))
    x0 = batch_x[0]
    y0_np = np.ascontiguousarray(batch_y[0])
    ones_np = np.ones((128, 2), dtype=np.float32)

    in_maps = []
    for c in range(NCORES):
        xl = batch_x[c * BL:(c + 1) * BL]
        xt_ext = np.empty((D, BL + EXT), dtype=np.float32)
        xt_ext[:, :BL] = xl.T
        xt_ext[:, BL] = x0
        xt_ext[:, BL + 1] = x0
        in_maps.append({
            "xt": rnd(xt_ext),
            "y_in": np.ascontiguousarray(batch_y[c * BL:(c + 1) * BL]),
            "w1t": w1t_np,
            "w2t": w2t_np,
            "b1s": b1s_np,
            "b1r": b1r_np,
            "b2": b2,
            "y0": y0_np,
            "onesd": ones_np,
        })
    return in_maps


def kernel(batch_x, batch_y, W1, b1, W2, b2, k, use_f32r=True, _res_out=None,
           _trace=False):
    from concourse.bass_utils import run_bass_kernel_spmd

    batch_x = np.asarray(batch_x, dtype=np.float32)
    batch_y = np.asarray(batch_y, dtype=np.float32)
    W1 = np.asarray(W1, dtype=np.float32)
    b1 = np.asarray(b1, dtype=np.float32)
    W2 = np.asarray(W2, dtype=np.float32)
    b2 = np.asarray(b2, dtype=np.float32)
    k = int(k)

    in_maps = _make_in_maps(batch_x, batch_y, W1, b1, W2, b2, use_f32r)

    nc = _get_nc(use_f32r)
    kwargs = {}
    if _trace:
        import tempfile
        kwargs = dict(trace=True, tmpdir=tempfile.mkdtemp(prefix="spine_tr_"))
    res = run_bass_kernel_spmd(nc, in_maps, core_ids=list(range(NCORES)),
                               **kwargs)
    if _res_out is not None:
        _res_out.append(res)
    r = res.results

    out = np.concatenate([r[c]["out_o"] for c in range(NCORES)], axis=0)
    h = np.concatenate(
        [np.ascontiguousarray(r[c]["ht_o"].T) for c in range(NCORES)], axis=0
    )

    rec_sum = sum(float(r[c]["rec_o"].sum()) for c in range(NCORES))
    reconstruction_loss = rec_sum / (B * OUT)

    colsum = np.zeros(H, dtype=np.float64)
    for c in range(NCORES):
        colsum += r[c]["cs_o"].T.ravel()
    sum_h = float(colsum.sum())
    sum_h2 = sum(float(r[c]["sc_o"][0, 1:3].sum()) for c in range(NCORES))
    psl_loss = (sum_h - sum_h2) / (B * H)

    asl_t = np.maximum(colsum / B - RHO_STAR, 0.0)
    asl_loss = float(np.sum(asl_t * asl_t)) / H

    d2_y = np.concatenate([r[c]["dy_o"].T.ravel() for c in range(NCORES)])
    d_y = np.sqrt(np.maximum(d2_y + D * EPS * EPS, 0.0))
    d2_h = np.concatenate(
        [r[c]["dh_o"][0, :BL] + (H * EPS * EPS - r[c]["dh_o"][0, BL])
         for c in range(NCORES)]
    )
    d_h = np.sqrt(np.maximum(d2_h, 0.0))
    d_y[0] = np.inf
    d_h[0] = np.inf
    sy = float(np.sort(np.partition(d_y, k)[:k]).sum())
    sh = float(np.sort(np.partition(d_h, k)[:k]).sum())
    local_loss = abs(sy - sh)

    total = reconstruction_loss + psl_loss + asl_loss + local_loss
    f32 = np.float32
    return (
        out,
        h,
        f32(total),
        f32(reconstruction_loss),
        f32(psl_loss),
        f32(asl_loss),
        f32(local_loss),
    )
